# revision 7
# baseline (speedup 1.0000x reference)
"""Bloom attention (separated QKV) — 8-core TRN2 Bass kernel.

Distribution: tensor-parallel over heads (2 heads/core). Per core:
  1. QKV projections (bf16): q^T,k^T in [d,s] layout bf16; v in [s,d]
     layout cast to fp8e4 (for DoubleRow ctx matmuls).
  2. Attention with transposed scores St[k,q] = k @ q^T (bf16), exp via
     ScalarE with alibi-C as per-partition bias -> P in fp8e4.
     ctx^T = v^T @ P and the softmax denominator (ones^T @ P) both as
     fp8 DoubleRow matmuls (K=256/instr, 0.5 cycles/row).
  3. ctx normalized by broadcast 1/den (DVE) -> fp8, chunked AllGather
     overlapped with remaining attention blocks.
  4. Output projection: fp8 DoubleRow with Wd*16 fp8; residual+bias
     (x16) DMA-preloaded into PSUM, matmuls accumulate on top, final
     DVE x(1/16) descale, DMA out.
Host side: transpose/slice/cast/scale packing only; concatenate the 8
output column-slices.
"""
import numpy as np
import ml_dtypes

import concourse.bass as bass
import concourse.bacc as bacc
import concourse.mybir as mybir
import concourse.tile as tile
import concourse.bass_utils as bass_utils

BF16 = ml_dtypes.bfloat16
E4M3 = ml_dtypes.float8_e4m3
N_CORES = 8
B, S, H = 2, 2048, 2048
NH, HD = 16, 128
HPC = NH // N_CORES          # heads per core
CI = HPC * HD                # per-core slice of H (256)
BS = B * S                   # 4096
INV_NORM = 1.0 / float(np.sqrt(HD))
EXP_SHIFT = 5.5              # global score shift so exp() fits fp8e4 (max 240)
# ctxn is stored as 4*ctx/den (via 1/4 folded into the den broadcast) and Wd
# as 4*Wd so both fp8 tensors sit in normal range; the dense epilogue applies
# the 1/16 descale + bd via one ACT op.
WD_SCALE = 4.0

JT = H // 128                # 16 contraction tiles for projections
SS_CHUNK = 512               # seq chunk for projections
N_CHUNKS = BS // SS_CHUNK    # 8
KT = S // 128                # 16 key tiles per batch
IT = H // 128                # 16 contraction tiles for dense
QBLK = 1024                  # attention/AG/dense block along seq
N_BLOCKS = BS // QBLK        # 4
NPAIR = KT // 2              # 8 DoubleRow k-pairs per attention group

F32 = mybir.dt.float32
BF = mybir.dt.bfloat16
FP8 = mybir.dt.float8e4
DR = mybir.MatmulPerfMode.DoubleRow

IT_ORDER = [*range(0, IT, 2), *range(1, IT, 2)]  # hi=0 tiles first


def _build():
    nc = bacc.Bacc("TRN2", target_bir_lowering=False, debug=False,
                   num_devices=N_CORES)

    # host-packed to exact SBUF layouts so every DMA is contiguous
    hsT = nc.dram_tensor("hsT", [128, N_CHUNKS, JT, SS_CHUNK], BF,
                         kind="ExternalInput").ap()
    wqT = nc.dram_tensor("wqT", [128, JT, CI], BF, kind="ExternalInput").ap()
    wkT = nc.dram_tensor("wkT", [128, JT, CI], BF, kind="ExternalInput").ap()
    wvT = nc.dram_tensor("wvT", [128, JT, CI], BF, kind="ExternalInput").ap()
    wdT = nc.dram_tensor("wdT", [128, IT, CI], FP8, kind="ExternalInput").ap()
    bq = nc.dram_tensor("bq", [CI, 1], F32, kind="ExternalInput").ap()
    bk = nc.dram_tensor("bk", [CI, 1], F32, kind="ExternalInput").ap()
    bv = nc.dram_tensor("bv", [1, CI], BF, kind="ExternalInput").ap()
    bd_f32 = nc.dram_tensor("bd", [CI, 1], F32, kind="ExternalInput").ap()
    ones8_d = nc.dram_tensor("ones8", [128, 2, 32], FP8,
                             kind="ExternalInput").ap()
    alibi = nc.dram_tensor("alibi", [B * HPC, S], F32, kind="ExternalInput").ap()
    residT = nc.dram_tensor("residT", [CI, BS], F32, kind="ExternalInput").ap()
    outT = nc.dram_tensor("outT", [CI, BS], F32, kind="ExternalOutput").ap()

    bounce = nc.dram_tensor("bounce", [N_BLOCKS, HPC, 128, QBLK], FP8,
                            kind="Internal").ap()
    gath = nc.dram_tensor("gath", [HPC, N_BLOCKS, N_CORES * 128, QBLK], FP8,
                          kind="Internal", addr_space="Shared").ap()

    with tile.TileContext(nc) as tc:
        with (
            tc.tile_pool(name="const", bufs=1) as constp,
            tc.tile_pool(name="qkv", bufs=1) as qkvp,
            tc.tile_pool(name="ctile", bufs=16) as ctp,
        ):
            # ---- phase 0: constants (phase-1 critical ones first) ----
            wq_sb = constp.tile([128, JT, CI], BF)
            wk_sb = constp.tile([128, JT, CI], BF)
            wv_sb = constp.tile([128, JT, CI], BF)
            nc.gpsimd.dma_start(wq_sb[:], wqT[:])
            nc.scalar.dma_start(wk_sb[:], wkT[:])
            # wv is emitted on the sync queue right after hs chunk 0
            bq_sb = constp.tile([128, HPC], F32)
            bk_sb = constp.tile([128, HPC], F32)
            for b_sb, b_dr in ((bq_sb, bq), (bk_sb, bk)):
                for hi in range(HPC):
                    nc.scalar.dma_start(b_sb[:, hi:hi + 1],
                                        b_dr[hi * 128:(hi + 1) * 128, :])
            bv_sb = constp.tile([1, CI], BF)
            nc.scalar.dma_start(bv_sb[:], bv[:])
            ones8_sb = constp.tile([128, 2, 32], FP8)
            nc.scalar.dma_start(ones8_sb[:], ones8_d[:])
            alibi_sb = constp.tile([128, B * HPC, KT], F32)
            nc.scalar.dma_start(
                alibi_sb[:], alibi.rearrange("r (kt p) -> p r kt", p=128))
            ones_row_bf = constp.tile([1, 128], BF)    # bcast lhsT (K=1, M=128)
            nc.vector.memset(ones_row_bf[:], 1.0)
            # den broadcast uses 1/WD_SCALE so reciprocal yields WD_SCALE/den
            quart_row_bf = constp.tile([1, 128], BF)
            nc.vector.memset(quart_row_bf[:], 1.0 / WD_SCALE)

            # persistent per-core activations
            qT_sb = qkvp.tile([128, HPC, BS], BF)      # [d, hi, ss]
            kT_sb = qkvp.tile([128, HPC, BS], BF)
            v_sb = qkvp.tile([128, BS // 128, CI], FP8)  # [ss%128, ss//128, i]

            # ---- phase 1: QKV projections (bf16) ----
            with (
                tc.tile_pool(name="hsb", bufs=3) as hsp,
                tc.tile_pool(name="p1psum", bufs=4,
                             space=bass.MemorySpace.PSUM) as p1p,
            ):
                for ch in range(N_CHUNKS):
                    s0 = ch * SS_CHUNK
                    hsb = hsp.tile([128, JT, SS_CHUNK], BF, name="hsb")
                    # chunk 0 on the sync queue so it lands in parallel
                    # with wq on gpsimd
                    (nc.sync if ch == 0 else nc.gpsimd).dma_start(
                        hsb[:], hsT[:, ch])
                    if ch == 0:
                        nc.sync.dma_start(wv_sb[:], wvT[:])
                    for w_sb, b_col, o_sb, scale in (
                        (wq_sb, bq_sb, qT_sb, INV_NORM),
                        (wk_sb, bk_sb, kT_sb, 1.0),
                    ):
                        for hi in range(HPC):
                            ps = p1p.tile([128, SS_CHUNK], F32, name="ps_qk")
                            for jt in range(JT):
                                nc.tensor.matmul(
                                    ps[:],
                                    w_sb[:, jt, hi * 128:(hi + 1) * 128],
                                    hsb[:, jt, :],
                                    start=(jt == 0), stop=(jt == JT - 1))
                            nc.scalar.activation(
                                o_sb[:, hi, s0:s0 + SS_CHUNK], ps[:],
                                mybir.ActivationFunctionType.Identity,
                                bias=b_col[:, hi:hi + 1], scale=scale)
                    for st in range(SS_CHUNK // 128):
                        ps = p1p.tile([128, CI], F32, name="ps_v")
                        nc.tensor.matmul(ps[:], ones_row_bf[:], bv_sb[:],
                                         start=True, stop=False)
                        for jt in range(JT):
                            nc.tensor.matmul(
                                ps[:],
                                hsb[:, jt, st * 128:(st + 1) * 128],
                                wv_sb[:, jt, :],
                                start=False, stop=(jt == JT - 1))
                        nc.scalar.copy(v_sb[:, ch * 4 + st, :], ps[:])

            # late consts (dense phase only)
            wd_sb = constp.tile([128, IT, CI], FP8)
            nc.sync.dma_start(wd_sb[:], wdT[:])
            bd_col = constp.tile([128, HPC], F32)
            for ci in range(HPC):
                nc.sync.dma_start(bd_col[:, ci:ci + 1],
                                  bd_f32[ci * 128:(ci + 1) * 128, :])

            # ---- phase 2+3: attention blocks + chunked AllGather ----
            with (
                tc.tile_pool(name="stp", bufs=2,
                             space=bass.MemorySpace.PSUM) as stp,
                tc.tile_pool(name="ptp", bufs=5) as ptp,
                tc.tile_pool(name="accp", bufs=1,
                             space=bass.MemorySpace.PSUM) as accp,
                tc.tile_pool(name="denp", bufs=1,
                             space=bass.MemorySpace.PSUM) as denp,
                tc.tile_pool(name="normp", bufs=2) as normp,
            ):
                LAG = 5
                pending_tail = [None]
                ctiles = {}

                def prefetch_ctiles(blk):
                    tiles = []
                    for j in range(NPAIR):
                        it_a, it_b = IT_ORDER[2 * j], IT_ORDER[2 * j + 1]
                        ct2 = ctp.tile([128, 2, QBLK], FP8, name="ctile")
                        for sl, it in ((0, it_a), (1, it_b)):
                            nc.gpsimd.dma_start(
                                ct2[:, sl, :],
                                gath[it % HPC, blk,
                                     (it // HPC) * 128:(it // HPC + 1) * 128,
                                     :])
                        tiles.append(ct2)
                    ctiles[blk] = tiles

                def flush_tail():
                    if pending_tail[0] is not None:
                        pending_tail[0]()
                        pending_tail[0] = None

                for blk in range(N_BLOCKS):
                    b, qh = divmod(blk, N_BLOCKS // B)
                    q0 = b * S + qh * QBLK
                    for hi in range(HPC):
                        bh = b * HPC + hi
                        ctx_ps = accp.tile([128, QBLK], F32, name="ctx_ps")
                        den_ps = denp.tile([128, 2, SS_CHUNK], F32,
                                           name="den_ps")
                        pts = []
                        consumed = [0]

                        def consume(p, ctx_ps=ctx_ps, den_ps=den_ps,
                                    pts=pts, b=b, hi=hi):
                            pt = pts[p]
                            for half in range(2):
                                hs_ = slice(half * SS_CHUNK,
                                            (half + 1) * SS_CHUNK)
                                # ctx^T += v_pair^T @ P_pair  (fp8 DoubleRow)
                                nc.tensor.matmul(
                                    ctx_ps[:, hs_],
                                    v_sb[:, (b * S) // 128 + 2 * p:
                                         (b * S) // 128 + 2 * p + 2,
                                         hi * 128:(hi + 1) * 128],
                                    pt[:, :, half, :],
                                    start=(p == 0), stop=(p == NPAIR - 1),
                                    perf_mode=DR)
                                # den += ones^T @ P_pair
                                nc.tensor.matmul(
                                    den_ps[:32, half, :],
                                    ones8_sb[:],
                                    pt[:, :, half, :],
                                    start=(p == 0), stop=(p == NPAIR - 1),
                                    perf_mode=DR)

                        for kt in range(KT):
                            k0 = b * S + kt * 128
                            st_ps = stp.tile([128, 2, SS_CHUNK], F32,
                                             name="st_ps")
                            for half in range(2):
                                nc.tensor.matmul(
                                    st_ps[:, half, :],
                                    kT_sb[:, hi, k0:k0 + 128],
                                    qT_sb[:, hi,
                                          q0 + half * SS_CHUNK:
                                          q0 + (half + 1) * SS_CHUNK],
                                    start=True, stop=True)
                            if kt % 2 == 0:
                                pt = ptp.tile([128, 2, 2, SS_CHUNK], FP8,
                                              name="pt")
                                pts.append(pt)
                            # q pre-scaled by INV_NORM in phase 1; bias is
                            # alibi (per key-position partition) - EXP_SHIFT
                            nc.scalar.activation(
                                pts[-1][:, kt % 2], st_ps[:],
                                mybir.ActivationFunctionType.Exp,
                                bias=alibi_sb[:, bh, kt:kt + 1])
                            if kt == 2:
                                flush_tail()
                            while consumed[0] < (kt - LAG + 1) // 2:
                                consume(consumed[0])
                                consumed[0] += 1
                        while consumed[0] < NPAIR:
                            consume(consumed[0])
                            consumed[0] += 1
                        # den PSUM -> SBUF right away so the den bank can be
                        # reused by the next group without waiting the tail
                        den_sb = normp.tile([1, QBLK], BF, name="den_sb")
                        nc.vector.tensor_copy(
                            den_sb[:],
                            den_ps[:1, :, :].rearrange("p a b -> p (a b)"))

                        def tail(ctx_ps=ctx_ps, den_sb=den_sb, blk=blk,
                                 hi=hi):
                            denb_ps = stp.tile([128, 2, SS_CHUNK], F32,
                                               name="st_ps")
                            for half in range(2):
                                nc.tensor.matmul(
                                    denb_ps[:, half, :], quart_row_bf[:],
                                    den_sb[:, half * SS_CHUNK:
                                           (half + 1) * SS_CHUNK],
                                    start=True, stop=True)
                            denb_sb = normp.tile([128, QBLK], F32,
                                                 name="denb_sb")
                            nc.vector.reciprocal_approx_fast(
                                denb_sb[:],
                                denb_ps[:].rearrange("p a b -> p (a b)"))
                            ctxn_sb = normp.tile([128, QBLK], FP8,
                                                 name="ctxn_sb")
                            nc.vector.tensor_mul(ctxn_sb[:], ctx_ps[:],
                                                 denb_sb[:])
                            nc.sync.dma_start(bounce[blk, hi], ctxn_sb[:])
                            nc.gpsimd.collective_compute(
                                "AllGather", mybir.AluOpType.bypass,
                                replica_groups=[list(range(N_CORES))],
                                ins=[bounce[blk, hi]],
                                outs=[gath[hi, blk]])
                            # ctile prefetches slot onto the gpsimd queue
                            # after the AGs they depend on (queue-ordered)
                            if hi == 0 and blk >= 1:
                                prefetch_ctiles(blk - 1)

                        pending_tail[0] = tail
                flush_tail()
                prefetch_ctiles(N_BLOCKS - 1)

            # ---- phase 4: output projection, fp8 DoubleRow. PSUM is
            # DMA-preloaded with WD_SCALE*(residual + bd); matmuls accumulate
            # WD_SCALE*Wd @ ctxn on top; DVE applies the 1/WD_SCALE descale.
            with (
                tc.tile_pool(name="dpsum", bufs=8,
                             space=bass.MemorySpace.PSUM) as dpp,
                tc.tile_pool(name="outp", bufs=10) as outp,
            ):
                NSC = QBLK // SS_CHUNK      # 2 seq chunks per block
                for blk in range(N_BLOCKS):
                    b, qh = divmod(blk, N_BLOCKS // B)
                    q0 = b * S + qh * QBLK
                    dps = [dpp.tile([128, SS_CHUNK], F32, name="dps")
                           for _ in range(HPC * NSC)]
                    rts = []
                    for ct in range(HPC):
                        for sc in range(NSC):
                            rt = outp.tile([128, SS_CHUNK], F32, name="rt")
                            nc.sync.dma_start(
                                rt[:],
                                residT[ct * 128:(ct + 1) * 128,
                                       q0 + sc * SS_CHUNK:
                                       q0 + (sc + 1) * SS_CHUNK])
                            rts.append(rt)
                    for j in range(NPAIR):
                        ct2 = ctiles[blk][j]
                        for ct in range(HPC):
                            for sc in range(NSC):
                                nc.tensor.matmul(
                                    dps[ct * NSC + sc][:],
                                    wd_sb[:, 2 * j:2 * j + 2,
                                          ct * 128:(ct + 1) * 128],
                                    ct2[:, :, sc * SS_CHUNK:
                                        (sc + 1) * SS_CHUNK],
                                    start=(j == 0), stop=(j == NPAIR - 1),
                                    perf_mode=DR)
                    for ct in range(HPC):
                        for sc in range(NSC):
                            c0 = ct * 128
                            s0_ = q0 + sc * SS_CHUNK
                            osb = outp.tile([128, SS_CHUNK], F32,
                                            name="osb")
                            nc.scalar.activation(
                                osb[:], dps[ct * NSC + sc][:],
                                mybir.ActivationFunctionType.Identity,
                                bias=bd_col[:, ct:ct + 1],
                                scale=1.0 / (WD_SCALE * WD_SCALE))
                            osb2 = outp.tile([128, SS_CHUNK], F32,
                                             name="osb2")
                            nc.vector.tensor_add(
                                osb2[:], osb[:], rts[ct * NSC + sc][:])
                            nc.sync.dma_start(
                                outT[c0:c0 + 128, s0_:s0_ + SS_CHUNK],
                                osb2[:])

    nc.compile()
    return nc


_NC = None


def _get_nc():
    global _NC
    if _NC is None:
        _NC = _build()
    return _NC


def _pack_w(W, sl, dtype=BF16, scale=1.0, order=None):
    # [H, CI] transposed slice -> SBUF layout [128, JT, CI], contiguous
    wT = np.asarray(W, np.float32)[sl].T * scale    # [H, CI]
    w = wT.reshape(JT, 128, CI)
    if order is not None:
        w = w[order]
    return np.ascontiguousarray(w.transpose(1, 0, 2)).astype(dtype)


def _prep_in_maps(hidden_states, residual, alibi, Wq, bq, Wk, bk, Wv, bv,
                  Wd, bd):
    hs = np.ascontiguousarray(np.asarray(hidden_states, np.float32)
                              .reshape(BS, H))
    # SBUF chunk layout [128, ch, jt, s]: element = hs[ch*512+s, jt*128+p]
    hs_pack = np.ascontiguousarray(
        hs.reshape(N_CHUNKS, SS_CHUNK, JT, 128).transpose(3, 0, 2, 1)
    ).astype(BF16)
    resid = np.asarray(residual, np.float32).reshape(BS, H)
    bd_f = np.asarray(bd, np.float32)
    alibi_r = np.asarray(alibi, np.float32).reshape(B, NH, S)
    ones8 = np.ones((128, 2, 32), E4M3)
    in_maps = []
    for c in range(N_CORES):
        sl = slice(c * CI, (c + 1) * CI)
        # alibi rows ordered (b, hi); EXP_SHIFT folded into the exp bias
        al = np.ascontiguousarray(
            alibi_r[:, c * HPC:(c + 1) * HPC, :].reshape(B * HPC, S)
            - EXP_SHIFT)
        residT = np.ascontiguousarray(resid[:, sl].T)
        in_maps.append({
            "hsT": hs_pack,
            "wqT": _pack_w(Wq, sl),
            "wkT": _pack_w(Wk, sl),
            "wvT": _pack_w(Wv, sl),
            "wdT": _pack_w(Wd, sl, dtype=E4M3, scale=WD_SCALE,
                           order=IT_ORDER),
            "bq": np.asarray(bq, np.float32)[sl].reshape(CI, 1),
            "bk": np.asarray(bk, np.float32)[sl].reshape(CI, 1),
            "bv": np.asarray(bv, np.float32)[sl].reshape(1, CI).astype(BF16),
            "ones8": ones8,
            "bd": bd_f[sl].reshape(CI, 1),
            "alibi": al,
            "residT": residT,
        })
    return in_maps


def run(trace=False, trace_cores=None, stitch_traces=False, **inputs):
    nc = _get_nc()
    in_maps = _prep_in_maps(**inputs)
    res = bass_utils.run_bass_kernel_spmd(
        nc, in_maps, core_ids=list(range(N_CORES)), trace=trace,
        trace_cores=trace_cores, stitch_traces=stitch_traces)
    full = np.empty((BS, H), np.float32)
    for c in range(N_CORES):
        full[:, c * CI:(c + 1) * CI] = res.results[c]["outT"].T
    return full.reshape(B, S, H), res


def kernel(**inputs):
    out, _ = run(trace=False, **inputs)
    return out


# revision 8
# speedup vs baseline: 1.2565x; 1.2565x over previous
"""Bloom attention (separated QKV) — 8-core TRN2 Bass kernel.

Distribution: tensor-parallel over heads (2 heads/core). Per core:
  1. QKV projections (bf16): q^T,k^T in [d,s] layout bf16; v in [s,d]
     layout cast to fp8e4 (for DoubleRow ctx matmuls).
  2. Attention with transposed scores St[k,q] = k @ q^T (bf16), exp via
     ScalarE with alibi-C as per-partition bias -> P in fp8e4.
     ctx^T = v^T @ P and the softmax denominator (ones^T @ P) both as
     fp8 DoubleRow matmuls (K=256/instr, 0.5 cycles/row).
  3. ctx normalized by broadcast 1/den (DVE) -> fp8, chunked AllGather
     overlapped with remaining attention blocks.
  4. Output projection: fp8 DoubleRow with Wd*16 fp8; residual+bias
     (x16) DMA-preloaded into PSUM, matmuls accumulate on top, final
     DVE x(1/16) descale, DMA out.
Host side: transpose/slice/cast/scale packing only; concatenate the 8
output column-slices.
"""
import numpy as np
import ml_dtypes

import concourse.bass as bass
import concourse.bacc as bacc
import concourse.mybir as mybir
import concourse.tile as tile
import concourse.bass_utils as bass_utils

BF16 = ml_dtypes.bfloat16
E4M3 = ml_dtypes.float8_e4m3
N_CORES = 8
B, S, H = 2, 2048, 2048
NH, HD = 16, 128
HPC = NH // N_CORES          # heads per core
CI = HPC * HD                # per-core slice of H (256)
BS = B * S                   # 4096
INV_NORM = 1.0 / float(np.sqrt(HD))
EXP_SHIFT = 5.5              # global score shift so exp() fits fp8e4 (max 240)
# ctxn is stored as 4*ctx/den (via 1/4 folded into the den broadcast) and Wd
# as 4*Wd so both fp8 tensors sit in normal range; the dense epilogue applies
# the 1/16 descale + bd via one ACT op.
WD_SCALE = 4.0

JT = H // 128                # 16 contraction tiles for projections
SS_CHUNK = 512               # seq chunk for projections
N_CHUNKS = BS // SS_CHUNK    # 8
KT = S // 128                # 16 key tiles per batch
IT = H // 128                # 16 contraction tiles for dense
QBLK = 1024                  # attention/AG/dense block along seq
N_BLOCKS = BS // QBLK        # 4
NPAIR = KT // 2              # 8 DoubleRow k-pairs per attention group

F32 = mybir.dt.float32
BF = mybir.dt.bfloat16
FP8 = mybir.dt.float8e4
DR = mybir.MatmulPerfMode.DoubleRow

IT_ORDER = [*range(0, IT, 2), *range(1, IT, 2)]  # hi=0 tiles first


def _build():
    nc = bacc.Bacc("TRN2", target_bir_lowering=False, debug=False,
                   num_devices=N_CORES)

    # host-packed to exact SBUF layouts so every DMA is contiguous
    hsT = nc.dram_tensor("hsT", [128, N_CHUNKS, JT, SS_CHUNK], BF,
                         kind="ExternalInput").ap()
    hs8T = nc.dram_tensor("hs8T", [128, N_CHUNKS, JT, SS_CHUNK], FP8,
                          kind="ExternalInput").ap()
    wqT = nc.dram_tensor("wqT", [128, JT, CI], BF, kind="ExternalInput").ap()
    wkT = nc.dram_tensor("wkT", [128, JT, CI], BF, kind="ExternalInput").ap()
    wvT = nc.dram_tensor("wvT", [128, JT, CI], FP8, kind="ExternalInput").ap()
    wdT = nc.dram_tensor("wdT", [128, IT, CI], FP8, kind="ExternalInput").ap()
    bq = nc.dram_tensor("bq", [CI, 1], F32, kind="ExternalInput").ap()
    bk = nc.dram_tensor("bk", [CI, 1], F32, kind="ExternalInput").ap()
    bv = nc.dram_tensor("bv", [1, CI], BF, kind="ExternalInput").ap()
    bd_f32 = nc.dram_tensor("bd", [CI, 1], F32, kind="ExternalInput").ap()
    ones8_d = nc.dram_tensor("ones8", [128, 2, 32], FP8,
                             kind="ExternalInput").ap()
    alibi = nc.dram_tensor("alibi", [B * HPC, S], F32, kind="ExternalInput").ap()
    residT = nc.dram_tensor("residT", [CI, BS], F32, kind="ExternalInput").ap()
    outT = nc.dram_tensor("outT", [CI, BS], F32, kind="ExternalOutput").ap()

    bounce = nc.dram_tensor("bounce", [N_BLOCKS, HPC, 128, QBLK], FP8,
                            kind="Internal").ap()
    gath = nc.dram_tensor("gath", [HPC, N_BLOCKS, N_CORES * 128, QBLK], FP8,
                          kind="Internal", addr_space="Shared").ap()

    with tile.TileContext(nc) as tc:
        with (
            tc.tile_pool(name="const", bufs=1) as constp,
            tc.tile_pool(name="qkv", bufs=1) as qkvp,
            tc.tile_pool(name="ctile", bufs=24) as ctp,
        ):
            # ---- phase 0: constants (phase-1 critical ones first) ----
            wq_sb = constp.tile([128, JT, CI], BF)
            wk_sb = constp.tile([128, JT, CI], BF)
            wv_sb = constp.tile([128, JT, CI], FP8)
            nc.gpsimd.dma_start(wq_sb[:, :JT // 2], wqT[:, :JT // 2])
            nc.gpsimd.dma_start(wq_sb[:, JT // 2:], wqT[:, JT // 2:])
            bq_sb = constp.tile([128, HPC], F32)
            bk_sb = constp.tile([128, HPC], F32)
            for b_sb, b_dr in ((bq_sb, bq), (bk_sb, bk)):
                for hi in range(HPC):
                    nc.scalar.dma_start(b_sb[:, hi:hi + 1],
                                        b_dr[hi * 128:(hi + 1) * 128, :])
            bv_sb = constp.tile([1, CI], BF)
            nc.scalar.dma_start(bv_sb[:], bv[:])
            ones8_sb = constp.tile([128, 2, 32], FP8)
            nc.scalar.dma_start(ones8_sb[:], ones8_d[:])
            alibi_sb = constp.tile([128, B * HPC, KT], F32)
            nc.scalar.dma_start(
                alibi_sb[:], alibi.rearrange("r (kt p) -> p r kt", p=128))
            nc.scalar.dma_start(wk_sb[:, :JT // 2], wkT[:, :JT // 2])
            nc.scalar.dma_start(wk_sb[:, JT // 2:], wkT[:, JT // 2:])
            nc.sync.dma_start(wv_sb[:], wvT[:])
            ones_row_bf = constp.tile([1, 128], BF)    # bcast lhsT (K=1, M=128)
            nc.vector.memset(ones_row_bf[:], 1.0)
            # den broadcast uses 1/WD_SCALE so reciprocal yields WD_SCALE/den
            quart_row_bf = constp.tile([1, 128], BF)
            nc.vector.memset(quart_row_bf[:], 1.0 / WD_SCALE)

            # persistent per-core activations
            qT_sb = qkvp.tile([128, HPC, BS], BF)      # [d, hi, ss]
            kT_sb = qkvp.tile([128, HPC, BS], BF)
            v_sb = qkvp.tile([128, BS // 128, CI], FP8)  # [ss%128, ss//128, i]

            # ---- phase 1: QKV projections (bf16) ----
            with (
                tc.tile_pool(name="hsb", bufs=2) as hsp,
                tc.tile_pool(name="hs8b", bufs=2) as hs8p,
                tc.tile_pool(name="p1psum", bufs=4,
                             space=bass.MemorySpace.PSUM) as p1p,
            ):
                for ch in range(N_CHUNKS):
                    s0 = ch * SS_CHUNK
                    hsb = hsp.tile([128, JT, SS_CHUNK], BF, name="hsb")
                    hs8b = hs8p.tile([128, JT, SS_CHUNK], FP8, name="hs8b")
                    # chunk 0 on the sync queue (in halves) so it lands in
                    # parallel with wq on gpsimd
                    q1 = nc.sync if ch == 0 else nc.gpsimd
                    q1.dma_start(hsb[:, :JT // 2], hsT[:, ch, :JT // 2])
                    q1.dma_start(hsb[:, JT // 2:], hsT[:, ch, JT // 2:])
                    nc.scalar.dma_start(hs8b[:], hs8T[:, ch])
                    for w_sb, b_col, o_sb, scale in (
                        (wq_sb, bq_sb, qT_sb, INV_NORM),
                        (wk_sb, bk_sb, kT_sb, 1.0),
                    ):
                        for hi in range(HPC):
                            ps = p1p.tile([128, SS_CHUNK], F32, name="ps_qk")
                            for jt in range(JT):
                                nc.tensor.matmul(
                                    ps[:],
                                    w_sb[:, jt, hi * 128:(hi + 1) * 128],
                                    hsb[:, jt, :],
                                    start=(jt == 0), stop=(jt == JT - 1))
                            nc.scalar.activation(
                                o_sb[:, hi, s0:s0 + SS_CHUNK], ps[:],
                                mybir.ActivationFunctionType.Identity,
                                bias=b_col[:, hi:hi + 1], scale=scale)
                    for st in range(SS_CHUNK // 128):
                        ps = p1p.tile([128, CI], F32, name="ps_v")
                        # bias preload (bv pre-scaled x WD_SCALE... no: x16
                        # on host to cancel the 1/16 evacuation descale)
                        nc.tensor.matmul(ps[:], ones_row_bf[:], bv_sb[:],
                                         start=True, stop=False)
                        for j in range(JT // 2):
                            nc.tensor.matmul(
                                ps[:],
                                hs8b[:, 2 * j:2 * j + 2,
                                     st * 128:(st + 1) * 128],
                                wv_sb[:, 2 * j:2 * j + 2, :],
                                start=False, stop=(j == JT // 2 - 1),
                                perf_mode=DR)
                        # 1/16 undoes the x16 host scaling of wv (and bv)
                        nc.scalar.activation(
                            v_sb[:, ch * 4 + st, :], ps[:],
                            mybir.ActivationFunctionType.Identity,
                            scale=1.0 / 16.0)

            # late consts (dense phase only)
            wd_sb = constp.tile([128, IT, CI], FP8)
            nc.sync.dma_start(wd_sb[:], wdT[:])
            bd_col = constp.tile([128, HPC], F32)
            for ci in range(HPC):
                nc.sync.dma_start(bd_col[:, ci:ci + 1],
                                  bd_f32[ci * 128:(ci + 1) * 128, :])

            # ---- phase 2+3: attention blocks + chunked AllGather ----
            with (
                tc.tile_pool(name="stp", bufs=2,
                             space=bass.MemorySpace.PSUM) as stp,
                tc.tile_pool(name="ptp", bufs=5) as ptp,
                tc.tile_pool(name="accp", bufs=1,
                             space=bass.MemorySpace.PSUM) as accp,
                tc.tile_pool(name="denp", bufs=1,
                             space=bass.MemorySpace.PSUM) as denp,
                tc.tile_pool(name="normp", bufs=2) as normp,
            ):
                LAG = 5
                pending_tail = [None]
                ctiles = {}

                def prefetch_ctiles(blk):
                    tiles = []
                    for j in range(NPAIR):
                        it_a = IT_ORDER[2 * j]
                        hi_j, rj = it_a % HPC, (it_a // HPC) // 2
                        ct2 = ctp.tile([128, 2, QBLK], FP8, name="ctile")
                        nc.gpsimd.dma_start(
                            ct2[:],
                            gath[hi_j, blk, rj * 256:(rj + 1) * 256, :]
                            .rearrange("(i p) q -> p i q", p=128))
                        tiles.append(ct2)
                    ctiles[blk] = tiles

                def flush_tail():
                    if pending_tail[0] is not None:
                        pending_tail[0]()
                        pending_tail[0] = None

                for blk in range(N_BLOCKS):
                    b, qh = divmod(blk, N_BLOCKS // B)
                    q0 = b * S + qh * QBLK
                    for hi in range(HPC):
                        bh = b * HPC + hi
                        ctx_ps = accp.tile([128, QBLK], F32, name="ctx_ps")
                        den_ps = denp.tile([128, 2, SS_CHUNK], F32,
                                           name="den_ps")
                        pts = []
                        consumed = [0]

                        def consume(p, ctx_ps=ctx_ps, den_ps=den_ps,
                                    pts=pts, b=b, hi=hi):
                            pt = pts[p]
                            for half in range(2):
                                hs_ = slice(half * SS_CHUNK,
                                            (half + 1) * SS_CHUNK)
                                # ctx^T += v_pair^T @ P_pair  (fp8 DoubleRow)
                                nc.tensor.matmul(
                                    ctx_ps[:, hs_],
                                    v_sb[:, (b * S) // 128 + 2 * p:
                                         (b * S) // 128 + 2 * p + 2,
                                         hi * 128:(hi + 1) * 128],
                                    pt[:, :, half, :],
                                    start=(p == 0), stop=(p == NPAIR - 1),
                                    perf_mode=DR)
                                # den += ones^T @ P_pair
                                nc.tensor.matmul(
                                    den_ps[:32, half, :],
                                    ones8_sb[:],
                                    pt[:, :, half, :],
                                    start=(p == 0), stop=(p == NPAIR - 1),
                                    perf_mode=DR)

                        for kt in range(KT):
                            k0 = b * S + kt * 128
                            st_ps = stp.tile([128, 2, SS_CHUNK], F32,
                                             name="st_ps")
                            for half in range(2):
                                nc.tensor.matmul(
                                    st_ps[:, half, :],
                                    kT_sb[:, hi, k0:k0 + 128],
                                    qT_sb[:, hi,
                                          q0 + half * SS_CHUNK:
                                          q0 + (half + 1) * SS_CHUNK],
                                    start=True, stop=True)
                            if kt % 2 == 0:
                                pt = ptp.tile([128, 2, 2, SS_CHUNK], FP8,
                                              name="pt")
                                pts.append(pt)
                            # q pre-scaled by INV_NORM in phase 1; bias is
                            # alibi (per key-position partition) - EXP_SHIFT
                            nc.scalar.activation(
                                pts[-1][:, kt % 2], st_ps[:],
                                mybir.ActivationFunctionType.Exp,
                                bias=alibi_sb[:, bh, kt:kt + 1])
                            if kt == 2:
                                flush_tail()
                            while consumed[0] < (kt - LAG + 1) // 2:
                                consume(consumed[0])
                                consumed[0] += 1
                        while consumed[0] < NPAIR:
                            consume(consumed[0])
                            consumed[0] += 1
                        # den PSUM -> SBUF right away so the den bank can be
                        # reused by the next group without waiting the tail
                        den_sb = normp.tile([1, QBLK], BF, name="den_sb")
                        nc.vector.tensor_copy(
                            den_sb[:],
                            den_ps[:1, :, :].rearrange("p a b -> p (a b)"))

                        def tail(ctx_ps=ctx_ps, den_sb=den_sb, blk=blk,
                                 hi=hi):
                            denb_ps = stp.tile([128, 2, SS_CHUNK], F32,
                                               name="st_ps")
                            for half in range(2):
                                nc.tensor.matmul(
                                    denb_ps[:, half, :], quart_row_bf[:],
                                    den_sb[:, half * SS_CHUNK:
                                           (half + 1) * SS_CHUNK],
                                    start=True, stop=True)
                            denb_sb = normp.tile([128, QBLK], F32,
                                                 name="denb_sb")
                            nc.vector.reciprocal_approx_fast(
                                denb_sb[:],
                                denb_ps[:].rearrange("p a b -> p (a b)"))
                            ctxn_sb = normp.tile([128, QBLK], FP8,
                                                 name="ctxn_sb")
                            nc.vector.tensor_mul(ctxn_sb[:], ctx_ps[:],
                                                 denb_sb[:])
                            nc.sync.dma_start(bounce[blk, hi], ctxn_sb[:])
                            nc.gpsimd.collective_compute(
                                "AllGather", mybir.AluOpType.bypass,
                                replica_groups=[list(range(N_CORES))],
                                ins=[bounce[blk, hi]],
                                outs=[gath[hi, blk]])
                            # ctile prefetches slot onto the gpsimd queue
                            # after the AGs they depend on (queue-ordered)
                            if hi == 1:
                                prefetch_ctiles(blk)

                        pending_tail[0] = tail
                flush_tail()

            # ---- phase 4: output projection, fp8 DoubleRow. PSUM is
            # DMA-preloaded with WD_SCALE*(residual + bd); matmuls accumulate
            # WD_SCALE*Wd @ ctxn on top; DVE applies the 1/WD_SCALE descale.
            with (
                tc.tile_pool(name="dpsum", bufs=8,
                             space=bass.MemorySpace.PSUM) as dpp,
                tc.tile_pool(name="outp", bufs=6) as outp,
            ):
                NSC = QBLK // SS_CHUNK      # 2 seq chunks per block
                for blk in range(N_BLOCKS):
                    b, qh = divmod(blk, N_BLOCKS // B)
                    q0 = b * S + qh * QBLK
                    dps = [dpp.tile([128, SS_CHUNK], F32, name="dps")
                           for _ in range(HPC * NSC)]
                    rts = []
                    for ct in range(HPC):
                        for sc in range(NSC):
                            rt = outp.tile([128, SS_CHUNK], F32, name="rt")
                            nc.sync.dma_start(
                                rt[:],
                                residT[ct * 128:(ct + 1) * 128,
                                       q0 + sc * SS_CHUNK:
                                       q0 + (sc + 1) * SS_CHUNK])
                            rts.append(rt)
                    for j in range(NPAIR):
                        ct2 = ctiles[blk][j]
                        for ct in range(HPC):
                            for sc in range(NSC):
                                nc.tensor.matmul(
                                    dps[ct * NSC + sc][:],
                                    wd_sb[:, 2 * j:2 * j + 2,
                                          ct * 128:(ct + 1) * 128],
                                    ct2[:, :, sc * SS_CHUNK:
                                        (sc + 1) * SS_CHUNK],
                                    start=(j == 0), stop=(j == NPAIR - 1),
                                    perf_mode=DR)
                    for ct in range(HPC):
                        for sc in range(NSC):
                            c0 = ct * 128
                            s0_ = q0 + sc * SS_CHUNK
                            osb = outp.tile([128, SS_CHUNK], F32,
                                            name="osb")
                            nc.scalar.activation(
                                osb[:], dps[ct * NSC + sc][:],
                                mybir.ActivationFunctionType.Identity,
                                bias=bd_col[:, ct:ct + 1],
                                scale=1.0 / (WD_SCALE * WD_SCALE))
                            osb2 = outp.tile([128, SS_CHUNK], F32,
                                             name="osb2")
                            nc.vector.tensor_add(
                                osb2[:], osb[:], rts[ct * NSC + sc][:])
                            nc.sync.dma_start(
                                outT[c0:c0 + 128, s0_:s0_ + SS_CHUNK],
                                osb2[:])

    nc.compile()
    return nc


_NC = None


def _get_nc():
    global _NC
    if _NC is None:
        _NC = _build()
    return _NC


def _pack_w(W, sl, dtype=BF16, scale=1.0, order=None):
    # [H, CI] transposed slice -> SBUF layout [128, JT, CI], contiguous
    wT = np.asarray(W, np.float32)[sl].T * scale    # [H, CI]
    w = wT.reshape(JT, 128, CI)
    if order is not None:
        w = w[order]
    return np.ascontiguousarray(w.transpose(1, 0, 2)).astype(dtype)


def _prep_in_maps(hidden_states, residual, alibi, Wq, bq, Wk, bk, Wv, bv,
                  Wd, bd):
    hs = np.ascontiguousarray(np.asarray(hidden_states, np.float32)
                              .reshape(BS, H))
    # SBUF chunk layout [128, ch, jt, s]: element = hs[ch*512+s, jt*128+p]
    hs_r = hs.reshape(N_CHUNKS, SS_CHUNK, JT, 128).transpose(3, 0, 2, 1)
    hs_pack = np.ascontiguousarray(hs_r).astype(BF16)
    hs8_pack = np.ascontiguousarray(hs_r).astype(E4M3)
    resid = np.asarray(residual, np.float32).reshape(BS, H)
    bd_f = np.asarray(bd, np.float32)
    alibi_r = np.asarray(alibi, np.float32).reshape(B, NH, S)
    ones8 = np.ones((128, 2, 32), E4M3)
    in_maps = []
    for c in range(N_CORES):
        sl = slice(c * CI, (c + 1) * CI)
        # alibi rows ordered (b, hi); EXP_SHIFT folded into the exp bias
        al = np.ascontiguousarray(
            alibi_r[:, c * HPC:(c + 1) * HPC, :].reshape(B * HPC, S)
            - EXP_SHIFT)
        residT = np.ascontiguousarray(resid[:, sl].T)
        in_maps.append({
            "hsT": hs_pack,
            "hs8T": hs8_pack,
            "wqT": _pack_w(Wq, sl),
            "wkT": _pack_w(Wk, sl),
            "wvT": _pack_w(Wv, sl, dtype=E4M3, scale=16.0),
            "wdT": _pack_w(Wd, sl, dtype=E4M3, scale=WD_SCALE,
                           order=IT_ORDER),
            "bq": np.asarray(bq, np.float32)[sl].reshape(CI, 1),
            "bk": np.asarray(bk, np.float32)[sl].reshape(CI, 1),
            "bv": (np.asarray(bv, np.float32)[sl].reshape(1, CI)
                   * 16.0).astype(BF16),
            "ones8": ones8,
            "bd": bd_f[sl].reshape(CI, 1),
            "alibi": al,
            "residT": residT,
        })
    return in_maps


def run(trace=False, trace_cores=None, stitch_traces=False, **inputs):
    nc = _get_nc()
    in_maps = _prep_in_maps(**inputs)
    res = bass_utils.run_bass_kernel_spmd(
        nc, in_maps, core_ids=list(range(N_CORES)), trace=trace,
        trace_cores=trace_cores, stitch_traces=stitch_traces)
    full = np.empty((BS, H), np.float32)
    for c in range(N_CORES):
        full[:, c * CI:(c + 1) * CI] = res.results[c]["outT"].T
    return full.reshape(B, S, H), res


def kernel(**inputs):
    out, _ = run(trace=False, **inputs)
    return out


# revision 9
# speedup vs baseline: 1.2691x; 1.0100x over previous
"""Bloom attention (separated QKV) — 8-core TRN2 Bass kernel.

Distribution: tensor-parallel over heads (2 heads/core). Per core:
  1. QKV projections (bf16): q^T,k^T in [d,s] layout bf16; v in [s,d]
     layout cast to fp8e4 (for DoubleRow ctx matmuls).
  2. Attention with transposed scores St[k,q] = k @ q^T (bf16), exp via
     ScalarE with alibi-C as per-partition bias -> P in fp8e4.
     ctx^T = v^T @ P and the softmax denominator (ones^T @ P) both as
     fp8 DoubleRow matmuls (K=256/instr, 0.5 cycles/row).
  3. ctx normalized by broadcast 1/den (DVE) -> fp8, chunked AllGather
     overlapped with remaining attention blocks.
  4. Output projection: fp8 DoubleRow with Wd*16 fp8; residual+bias
     (x16) DMA-preloaded into PSUM, matmuls accumulate on top, final
     DVE x(1/16) descale, DMA out.
Host side: transpose/slice/cast/scale packing only; concatenate the 8
output column-slices.
"""
import numpy as np
import ml_dtypes

import concourse.bass as bass
import concourse.bacc as bacc
import concourse.mybir as mybir
import concourse.tile as tile
import concourse.bass_utils as bass_utils

BF16 = ml_dtypes.bfloat16
E4M3 = ml_dtypes.float8_e4m3
N_CORES = 8
B, S, H = 2, 2048, 2048
NH, HD = 16, 128
HPC = NH // N_CORES          # heads per core
CI = HPC * HD                # per-core slice of H (256)
BS = B * S                   # 4096
INV_NORM = 1.0 / float(np.sqrt(HD))
EXP_SHIFT = 5.5              # global score shift so exp() fits fp8e4 (max 240)
# ctxn is stored as 4*ctx/den (via 1/4 folded into the den broadcast) and Wd
# as 4*Wd so both fp8 tensors sit in normal range; the dense epilogue applies
# the 1/16 descale + bd via one ACT op.
WD_SCALE = 4.0

JT = H // 128                # 16 contraction tiles for projections
SS_CHUNK = 512               # seq chunk for projections
N_CHUNKS = BS // SS_CHUNK    # 8
KT = S // 128                # 16 key tiles per batch
IT = H // 128                # 16 contraction tiles for dense
QBLK = 1024                  # attention/AG/dense block along seq
N_BLOCKS = BS // QBLK        # 4
NPAIR = KT // 2              # 8 DoubleRow k-pairs per attention group

F32 = mybir.dt.float32
BF = mybir.dt.bfloat16
FP8 = mybir.dt.float8e4
DR = mybir.MatmulPerfMode.DoubleRow

IT_ORDER = [*range(0, IT, 2), *range(1, IT, 2)]  # hi=0 tiles first


def _build():
    nc = bacc.Bacc("TRN2", target_bir_lowering=False, debug=False,
                   num_devices=N_CORES)

    # host-packed to exact SBUF layouts so every DMA is contiguous
    hsT = nc.dram_tensor("hsT", [128, N_CHUNKS, JT, SS_CHUNK], BF,
                         kind="ExternalInput").ap()
    hs8T = nc.dram_tensor("hs8T", [128, N_CHUNKS, JT, SS_CHUNK], FP8,
                          kind="ExternalInput").ap()
    wqT = nc.dram_tensor("wqT", [128, JT, CI], BF, kind="ExternalInput").ap()
    wkT = nc.dram_tensor("wkT", [128, JT, CI], BF, kind="ExternalInput").ap()
    wvT = nc.dram_tensor("wvT", [128, JT, CI], FP8, kind="ExternalInput").ap()
    wdT = nc.dram_tensor("wdT", [128, IT, CI], FP8, kind="ExternalInput").ap()
    bq = nc.dram_tensor("bq", [CI, 1], F32, kind="ExternalInput").ap()
    bk = nc.dram_tensor("bk", [CI, 1], F32, kind="ExternalInput").ap()
    bv = nc.dram_tensor("bv", [1, CI], BF, kind="ExternalInput").ap()
    bd_f32 = nc.dram_tensor("bd", [CI, 1], F32, kind="ExternalInput").ap()
    ones8_d = nc.dram_tensor("ones8", [128, 2, 32], FP8,
                             kind="ExternalInput").ap()
    alibi = nc.dram_tensor("alibi", [B * HPC, S], F32, kind="ExternalInput").ap()
    residT = nc.dram_tensor("residT", [CI, BS], F32, kind="ExternalInput").ap()
    outT = nc.dram_tensor("outT", [CI, BS], F32, kind="ExternalOutput").ap()

    bounce = nc.dram_tensor("bounce", [N_BLOCKS, HPC, 128, QBLK], FP8,
                            kind="Internal").ap()
    gath = nc.dram_tensor("gath", [HPC, N_BLOCKS, N_CORES * 128, QBLK], FP8,
                          kind="Internal", addr_space="Shared").ap()

    with tile.TileContext(nc) as tc:
        with (
            tc.tile_pool(name="const", bufs=1) as constp,
            tc.tile_pool(name="qkv", bufs=1) as qkvp,
            tc.tile_pool(name="ctile", bufs=24) as ctp,
        ):
            # ---- phase 0: constants (phase-1 critical ones first) ----
            wq_sb = constp.tile([128, JT, CI], BF)
            wk_sb = constp.tile([128, JT, CI], BF)
            wv_sb = constp.tile([128, JT, CI], FP8)
            nc.gpsimd.dma_start(wq_sb[:, :JT // 2], wqT[:, :JT // 2])
            nc.gpsimd.dma_start(wq_sb[:, JT // 2:], wqT[:, JT // 2:])
            bq_sb = constp.tile([128, HPC], F32)
            bk_sb = constp.tile([128, HPC], F32)
            for b_sb, b_dr in ((bq_sb, bq), (bk_sb, bk)):
                for hi in range(HPC):
                    nc.scalar.dma_start(b_sb[:, hi:hi + 1],
                                        b_dr[hi * 128:(hi + 1) * 128, :])
            bv_sb = constp.tile([1, CI], BF)
            nc.scalar.dma_start(bv_sb[:], bv[:])
            ones8_sb = constp.tile([128, 2, 32], FP8)
            nc.scalar.dma_start(ones8_sb[:], ones8_d[:])
            alibi_sb = constp.tile([128, B * HPC, KT], F32)
            nc.scalar.dma_start(
                alibi_sb[:], alibi.rearrange("r (kt p) -> p r kt", p=128))
            nc.sync.dma_start(wv_sb[:], wvT[:])
            ones_row_bf = constp.tile([1, 128], BF)    # bcast lhsT (K=1, M=128)
            nc.vector.memset(ones_row_bf[:], 1.0)
            # den broadcast uses 1/WD_SCALE so reciprocal yields WD_SCALE/den
            quart_row_bf = constp.tile([1, 128], BF)
            nc.vector.memset(quart_row_bf[:], 1.0 / WD_SCALE)

            # persistent per-core activations
            qT_sb = qkvp.tile([128, HPC, BS], BF)      # [d, hi, ss]
            kT_sb = qkvp.tile([128, HPC, BS], BF)
            v_sb = qkvp.tile([128, BS // 128, CI], FP8)  # [ss%128, ss//128, i]

            # ---- phase 1: QKV projections (bf16) ----
            with (
                tc.tile_pool(name="hsb", bufs=2) as hsp,
                tc.tile_pool(name="hs8b", bufs=2) as hs8p,
                tc.tile_pool(name="p1psum", bufs=4,
                             space=bass.MemorySpace.PSUM) as p1p,
            ):
                for ch in range(N_CHUNKS):
                    s0 = ch * SS_CHUNK
                    hsb = hsp.tile([128, JT, SS_CHUNK], BF, name="hsb")
                    hs8b = hs8p.tile([128, JT, SS_CHUNK], FP8, name="hs8b")
                    # chunk 0 on the scalar queue (in halves) so it lands
                    # in parallel with wq on gpsimd; wk follows it
                    q1 = nc.scalar if ch == 0 else nc.gpsimd
                    q1.dma_start(hsb[:, :JT // 2], hsT[:, ch, :JT // 2])
                    q1.dma_start(hsb[:, JT // 2:], hsT[:, ch, JT // 2:])
                    if ch == 0:
                        nc.scalar.dma_start(wk_sb[:, :JT // 2],
                                            wkT[:, :JT // 2])
                        nc.scalar.dma_start(hs8b[:], hs8T[:, ch])
                        nc.scalar.dma_start(wk_sb[:, JT // 2:],
                                            wkT[:, JT // 2:])
                    else:
                        nc.scalar.dma_start(hs8b[:], hs8T[:, ch])
                    for w_sb, b_col, o_sb, scale in (
                        (wq_sb, bq_sb, qT_sb, INV_NORM),
                        (wk_sb, bk_sb, kT_sb, 1.0),
                    ):
                        for hi in range(HPC):
                            ps = p1p.tile([128, SS_CHUNK], F32, name="ps_qk")
                            for jt in range(JT):
                                nc.tensor.matmul(
                                    ps[:],
                                    w_sb[:, jt, hi * 128:(hi + 1) * 128],
                                    hsb[:, jt, :],
                                    start=(jt == 0), stop=(jt == JT - 1))
                            nc.scalar.activation(
                                o_sb[:, hi, s0:s0 + SS_CHUNK], ps[:],
                                mybir.ActivationFunctionType.Identity,
                                bias=b_col[:, hi:hi + 1], scale=scale)
                    for st in range(SS_CHUNK // 128):
                        ps = p1p.tile([128, CI], F32, name="ps_v")
                        # bias preload (bv pre-scaled x WD_SCALE... no: x16
                        # on host to cancel the 1/16 evacuation descale)
                        nc.tensor.matmul(ps[:], ones_row_bf[:], bv_sb[:],
                                         start=True, stop=False)
                        for j in range(JT // 2):
                            nc.tensor.matmul(
                                ps[:],
                                hs8b[:, 2 * j:2 * j + 2,
                                     st * 128:(st + 1) * 128],
                                wv_sb[:, 2 * j:2 * j + 2, :],
                                start=False, stop=(j == JT // 2 - 1),
                                perf_mode=DR)
                        # 1/16 undoes the x16 host scaling of wv (and bv)
                        nc.scalar.activation(
                            v_sb[:, ch * 4 + st, :], ps[:],
                            mybir.ActivationFunctionType.Identity,
                            scale=1.0 / 16.0)

            # late consts (dense phase only)
            wd_sb = constp.tile([128, IT, CI], FP8)
            nc.sync.dma_start(wd_sb[:], wdT[:])
            bd_col = constp.tile([128, HPC], F32)
            for ci in range(HPC):
                nc.sync.dma_start(bd_col[:, ci:ci + 1],
                                  bd_f32[ci * 128:(ci + 1) * 128, :])

            # ---- phase 2+3: attention blocks + chunked AllGather ----
            with (
                tc.tile_pool(name="stp", bufs=2,
                             space=bass.MemorySpace.PSUM) as stp,
                tc.tile_pool(name="ptp", bufs=5) as ptp,
                tc.tile_pool(name="accp", bufs=1,
                             space=bass.MemorySpace.PSUM) as accp,
                tc.tile_pool(name="denp", bufs=1,
                             space=bass.MemorySpace.PSUM) as denp,
                tc.tile_pool(name="normp", bufs=4) as normp,
            ):
                LAG = 5
                pending_tail = [None]
                ctiles = {}

                def prefetch_ctiles(blk):
                    tiles = []
                    for j in range(NPAIR):
                        it_a = IT_ORDER[2 * j]
                        hi_j, rj = it_a % HPC, (it_a // HPC) // 2
                        ct2 = ctp.tile([128, 2, QBLK], FP8, name="ctile")
                        nc.gpsimd.dma_start(
                            ct2[:],
                            gath[hi_j, blk, rj * 256:(rj + 1) * 256, :]
                            .rearrange("(i p) q -> p i q", p=128))
                        tiles.append(ct2)
                    ctiles[blk] = tiles

                def flush_tail():
                    if pending_tail[0] is not None:
                        pending_tail[0]()
                        pending_tail[0] = None

                for blk in range(N_BLOCKS):
                    b, qh = divmod(blk, N_BLOCKS // B)
                    q0 = b * S + qh * QBLK
                    for hi in range(HPC):
                        bh = b * HPC + hi
                        ctx_ps = accp.tile([128, QBLK], F32, name="ctx_ps")
                        den_ps = denp.tile([128, 2, SS_CHUNK], F32,
                                           name="den_ps")
                        pts = []
                        consumed = [0]

                        def consume(p, ctx_ps=ctx_ps, den_ps=den_ps,
                                    pts=pts, b=b, hi=hi):
                            pt = pts[p]
                            for half in range(2):
                                hs_ = slice(half * SS_CHUNK,
                                            (half + 1) * SS_CHUNK)
                                # ctx^T += v_pair^T @ P_pair  (fp8 DoubleRow)
                                nc.tensor.matmul(
                                    ctx_ps[:, hs_],
                                    v_sb[:, (b * S) // 128 + 2 * p:
                                         (b * S) // 128 + 2 * p + 2,
                                         hi * 128:(hi + 1) * 128],
                                    pt[:, :, half, :],
                                    start=(p == 0), stop=(p == NPAIR - 1),
                                    perf_mode=DR)
                                # den += ones^T @ P_pair
                                nc.tensor.matmul(
                                    den_ps[:32, half, :],
                                    ones8_sb[:],
                                    pt[:, :, half, :],
                                    start=(p == 0), stop=(p == NPAIR - 1),
                                    perf_mode=DR)

                        for kt in range(KT):
                            k0 = b * S + kt * 128
                            st_ps = stp.tile([128, 2, SS_CHUNK], F32,
                                             name="st_ps")
                            for half in range(2):
                                nc.tensor.matmul(
                                    st_ps[:, half, :],
                                    kT_sb[:, hi, k0:k0 + 128],
                                    qT_sb[:, hi,
                                          q0 + half * SS_CHUNK:
                                          q0 + (half + 1) * SS_CHUNK],
                                    start=True, stop=True)
                            if kt % 2 == 0:
                                pt = ptp.tile([128, 2, 2, SS_CHUNK], FP8,
                                              name="pt")
                                pts.append(pt)
                            # q pre-scaled by INV_NORM in phase 1; bias is
                            # alibi (per key-position partition) - EXP_SHIFT
                            nc.scalar.activation(
                                pts[-1][:, kt % 2], st_ps[:],
                                mybir.ActivationFunctionType.Exp,
                                bias=alibi_sb[:, bh, kt:kt + 1])
                            if kt == 2:
                                flush_tail()
                            while consumed[0] < (kt - LAG + 1) // 2:
                                consume(consumed[0])
                                consumed[0] += 1
                        while consumed[0] < NPAIR:
                            consume(consumed[0])
                            consumed[0] += 1
                        # den PSUM -> SBUF right away so the den bank can be
                        # reused by the next group without waiting the tail
                        den_sb = normp.tile([1, QBLK], BF, name="den_sb")
                        nc.vector.tensor_copy(
                            den_sb[:],
                            den_ps[:1, :, :].rearrange("p a b -> p (a b)"))

                        def tail(ctx_ps=ctx_ps, den_sb=den_sb, blk=blk,
                                 hi=hi):
                            denb_ps = stp.tile([128, 2, SS_CHUNK], F32,
                                               name="st_ps")
                            for half in range(2):
                                nc.tensor.matmul(
                                    denb_ps[:, half, :], quart_row_bf[:],
                                    den_sb[:, half * SS_CHUNK:
                                           (half + 1) * SS_CHUNK],
                                    start=True, stop=True)
                            denb_sb = normp.tile([128, QBLK], F32,
                                                 name="denb_sb")
                            nc.vector.reciprocal_approx_fast(
                                denb_sb[:],
                                denb_ps[:].rearrange("p a b -> p (a b)"))
                            ctxn_sb = normp.tile([128, QBLK], FP8,
                                                 name="ctxn_sb")
                            nc.vector.tensor_mul(ctxn_sb[:], ctx_ps[:],
                                                 denb_sb[:])
                            nc.sync.dma_start(bounce[blk, hi], ctxn_sb[:])
                            nc.gpsimd.collective_compute(
                                "AllGather", mybir.AluOpType.bypass,
                                replica_groups=[list(range(N_CORES))],
                                ins=[bounce[blk, hi]],
                                outs=[gath[hi, blk]])
                            # ctile prefetches slot onto the gpsimd queue
                            # after the AGs they depend on (queue-ordered)
                            if hi == 1:
                                prefetch_ctiles(blk)

                        pending_tail[0] = tail
                flush_tail()

            # ---- phase 4: output projection, fp8 DoubleRow. PSUM is
            # DMA-preloaded with WD_SCALE*(residual + bd); matmuls accumulate
            # WD_SCALE*Wd @ ctxn on top; DVE applies the 1/WD_SCALE descale.
            with (
                tc.tile_pool(name="dpsum", bufs=8,
                             space=bass.MemorySpace.PSUM) as dpp,
                tc.tile_pool(name="outp", bufs=8) as outp,
            ):
                NSC = QBLK // SS_CHUNK      # 2 seq chunks per block
                for blk in range(N_BLOCKS):
                    b, qh = divmod(blk, N_BLOCKS // B)
                    q0 = b * S + qh * QBLK
                    dps = [dpp.tile([128, SS_CHUNK], F32, name="dps")
                           for _ in range(HPC * NSC)]
                    rts = []
                    for ct in range(HPC):
                        for sc in range(NSC):
                            rt = outp.tile([128, SS_CHUNK], F32, name="rt")
                            nc.sync.dma_start(
                                rt[:],
                                residT[ct * 128:(ct + 1) * 128,
                                       q0 + sc * SS_CHUNK:
                                       q0 + (sc + 1) * SS_CHUNK])
                            rts.append(rt)
                    for j in range(NPAIR):
                        ct2 = ctiles[blk][j]
                        for ct in range(HPC):
                            for sc in range(NSC):
                                nc.tensor.matmul(
                                    dps[ct * NSC + sc][:],
                                    wd_sb[:, 2 * j:2 * j + 2,
                                          ct * 128:(ct + 1) * 128],
                                    ct2[:, :, sc * SS_CHUNK:
                                        (sc + 1) * SS_CHUNK],
                                    start=(j == 0), stop=(j == NPAIR - 1),
                                    perf_mode=DR)
                    for ct in range(HPC):
                        for sc in range(NSC):
                            c0 = ct * 128
                            s0_ = q0 + sc * SS_CHUNK
                            osb = outp.tile([128, SS_CHUNK], F32,
                                            name="osb")
                            nc.scalar.activation(
                                osb[:], dps[ct * NSC + sc][:],
                                mybir.ActivationFunctionType.Identity,
                                bias=bd_col[:, ct:ct + 1],
                                scale=1.0 / (WD_SCALE * WD_SCALE))
                            nc.vector.tensor_add(
                                osb[:], osb[:], rts[ct * NSC + sc][:])
                            nc.sync.dma_start(
                                outT[c0:c0 + 128, s0_:s0_ + SS_CHUNK],
                                osb[:])

    nc.compile()
    return nc


_NC = None


def _get_nc():
    global _NC
    if _NC is None:
        _NC = _build()
    return _NC


def _pack_w(W, sl, dtype=BF16, scale=1.0, order=None):
    # [H, CI] transposed slice -> SBUF layout [128, JT, CI], contiguous
    wT = np.asarray(W, np.float32)[sl].T * scale    # [H, CI]
    w = wT.reshape(JT, 128, CI)
    if order is not None:
        w = w[order]
    return np.ascontiguousarray(w.transpose(1, 0, 2)).astype(dtype)


def _prep_in_maps(hidden_states, residual, alibi, Wq, bq, Wk, bk, Wv, bv,
                  Wd, bd):
    hs = np.ascontiguousarray(np.asarray(hidden_states, np.float32)
                              .reshape(BS, H))
    # SBUF chunk layout [128, ch, jt, s]: element = hs[ch*512+s, jt*128+p]
    hs_r = hs.reshape(N_CHUNKS, SS_CHUNK, JT, 128).transpose(3, 0, 2, 1)
    hs_pack = np.ascontiguousarray(hs_r).astype(BF16)
    hs8_pack = np.ascontiguousarray(hs_r).astype(E4M3)
    resid = np.asarray(residual, np.float32).reshape(BS, H)
    bd_f = np.asarray(bd, np.float32)
    alibi_r = np.asarray(alibi, np.float32).reshape(B, NH, S)
    ones8 = np.ones((128, 2, 32), E4M3)
    in_maps = []
    for c in range(N_CORES):
        sl = slice(c * CI, (c + 1) * CI)
        # alibi rows ordered (b, hi); EXP_SHIFT folded into the exp bias
        al = np.ascontiguousarray(
            alibi_r[:, c * HPC:(c + 1) * HPC, :].reshape(B * HPC, S)
            - EXP_SHIFT)
        residT = np.ascontiguousarray(resid[:, sl].T)
        in_maps.append({
            "hsT": hs_pack,
            "hs8T": hs8_pack,
            "wqT": _pack_w(Wq, sl),
            "wkT": _pack_w(Wk, sl),
            "wvT": _pack_w(Wv, sl, dtype=E4M3, scale=16.0),
            "wdT": _pack_w(Wd, sl, dtype=E4M3, scale=WD_SCALE,
                           order=IT_ORDER),
            "bq": np.asarray(bq, np.float32)[sl].reshape(CI, 1),
            "bk": np.asarray(bk, np.float32)[sl].reshape(CI, 1),
            "bv": (np.asarray(bv, np.float32)[sl].reshape(1, CI)
                   * 16.0).astype(BF16),
            "ones8": ones8,
            "bd": bd_f[sl].reshape(CI, 1),
            "alibi": al,
            "residT": residT,
        })
    return in_maps


def run(trace=False, trace_cores=None, stitch_traces=False, **inputs):
    nc = _get_nc()
    in_maps = _prep_in_maps(**inputs)
    res = bass_utils.run_bass_kernel_spmd(
        nc, in_maps, core_ids=list(range(N_CORES)), trace=trace,
        trace_cores=trace_cores, stitch_traces=stitch_traces)
    full = np.empty((BS, H), np.float32)
    for c in range(N_CORES):
        full[:, c * CI:(c + 1) * CI] = res.results[c]["outT"].T
    return full.reshape(B, S, H), res


def kernel(**inputs):
    out, _ = run(trace=False, **inputs)
    return out


# revision 11
# speedup vs baseline: 1.3001x; 1.0244x over previous
"""Bloom attention (separated QKV) — 8-core TRN2 Bass kernel.

Distribution: tensor-parallel over heads (2 heads/core). Per core:
  1. QKV projections (bf16): q^T,k^T in [d,s] layout bf16; v in [s,d]
     layout cast to fp8e4 (for DoubleRow ctx matmuls).
  2. Attention with transposed scores St[k,q] = k @ q^T (bf16), exp via
     ScalarE with alibi-C as per-partition bias -> P in fp8e4.
     ctx^T = v^T @ P and the softmax denominator (ones^T @ P) both as
     fp8 DoubleRow matmuls (K=256/instr, 0.5 cycles/row).
  3. ctx normalized by broadcast 1/den (DVE) -> fp8, chunked AllGather
     overlapped with remaining attention blocks.
  4. Output projection: fp8 DoubleRow with Wd*16 fp8; residual+bias
     (x16) DMA-preloaded into PSUM, matmuls accumulate on top, final
     DVE x(1/16) descale, DMA out.
Host side: transpose/slice/cast/scale packing only; concatenate the 8
output column-slices.
"""
import numpy as np
import ml_dtypes

import concourse.bass as bass
import concourse.bacc as bacc
import concourse.mybir as mybir
import concourse.tile as tile
import concourse.bass_utils as bass_utils

BF16 = ml_dtypes.bfloat16
E4M3 = ml_dtypes.float8_e4m3
N_CORES = 8
B, S, H = 2, 2048, 2048
NH, HD = 16, 128
HPC = NH // N_CORES          # heads per core
CI = HPC * HD                # per-core slice of H (256)
BS = B * S                   # 4096
INV_NORM = 1.0 / float(np.sqrt(HD))
EXP_SHIFT = 5.5              # global score shift so exp() fits fp8e4 (max 240)
# ctxn is stored as 4*ctx/den (via 1/4 folded into the den broadcast) and Wd
# as 4*Wd so both fp8 tensors sit in normal range; the dense epilogue applies
# the 1/16 descale + bd via one ACT op.
WD_SCALE = 4.0

JT = H // 128                # 16 contraction tiles for projections
SS_CHUNK = 512               # seq chunk for projections
N_CHUNKS = BS // SS_CHUNK    # 8
KT = S // 128                # 16 key tiles per batch
IT = H // 128                # 16 contraction tiles for dense
QBLK = 1024                  # attention/AG/dense block along seq
N_BLOCKS = BS // QBLK        # 4
NPAIR = KT // 2              # 8 DoubleRow k-pairs per attention group

F32 = mybir.dt.float32
BF = mybir.dt.bfloat16
FP8 = mybir.dt.float8e4
DR = mybir.MatmulPerfMode.DoubleRow

IT_ORDER = [*range(0, IT, 2), *range(1, IT, 2)]  # hi=0 tiles first


def _build():
    nc = bacc.Bacc("TRN2", target_bir_lowering=False, debug=False,
                   num_devices=N_CORES)

    # host-packed to exact SBUF layouts so every DMA is contiguous
    hsT = nc.dram_tensor("hsT", [128, N_CHUNKS, JT, SS_CHUNK], BF,
                         kind="ExternalInput").ap()
    hs8T = nc.dram_tensor("hs8T", [128, N_CHUNKS, JT, SS_CHUNK], FP8,
                          kind="ExternalInput").ap()
    wqT = nc.dram_tensor("wqT", [128, JT, CI], BF, kind="ExternalInput").ap()
    wkT = nc.dram_tensor("wkT", [128, JT, CI], BF, kind="ExternalInput").ap()
    wvT = nc.dram_tensor("wvT", [128, JT, CI], FP8, kind="ExternalInput").ap()
    wdT = nc.dram_tensor("wdT", [128, IT, CI], FP8, kind="ExternalInput").ap()
    bq = nc.dram_tensor("bq", [CI, 1], F32, kind="ExternalInput").ap()
    bk = nc.dram_tensor("bk", [CI, 1], F32, kind="ExternalInput").ap()
    bv = nc.dram_tensor("bv", [1, CI], BF, kind="ExternalInput").ap()
    bd_f32 = nc.dram_tensor("bd", [CI, 1], F32, kind="ExternalInput").ap()
    ones8_d = nc.dram_tensor("ones8", [128, 2, 32], FP8,
                             kind="ExternalInput").ap()
    alibi = nc.dram_tensor("alibi", [128, B * HPC, KT], F32,
                           kind="ExternalInput").ap()
    residT = nc.dram_tensor("residT", [CI, BS], F32, kind="ExternalInput").ap()
    outT = nc.dram_tensor("outT", [CI, BS], F32, kind="ExternalOutput").ap()

    bounce = nc.dram_tensor("bounce", [N_BLOCKS, HPC, 128, QBLK], FP8,
                            kind="Internal").ap()
    gath = nc.dram_tensor("gath", [HPC, N_BLOCKS, N_CORES * 128, QBLK], FP8,
                          kind="Internal", addr_space="Shared").ap()

    with tile.TileContext(nc) as tc:
        with (
            tc.tile_pool(name="const", bufs=1) as constp,
            tc.tile_pool(name="qkv", bufs=1) as qkvp,
            tc.tile_pool(name="ctile", bufs=24) as ctp,
        ):
            # ---- phase 0: constants (phase-1 critical ones first) ----
            wq_sb = constp.tile([128, JT, CI], BF)
            wk_sb = constp.tile([128, JT, CI], BF)
            wv_sb = constp.tile([128, JT, CI], FP8)
            nc.gpsimd.dma_start(wq_sb[:, :JT // 2], wqT[:, :JT // 2])
            nc.gpsimd.dma_start(wq_sb[:, JT // 2:], wqT[:, JT // 2:])
            nc.sync.dma_start(wv_sb[:], wvT[:])
            bq_sb = constp.tile([128, HPC], F32)
            bk_sb = constp.tile([128, HPC], F32)
            for b_sb, b_dr in ((bq_sb, bq), (bk_sb, bk)):
                for hi in range(HPC):
                    nc.sync.dma_start(b_sb[:, hi:hi + 1],
                                      b_dr[hi * 128:(hi + 1) * 128, :])
            bv_sb = constp.tile([1, CI], BF)
            nc.sync.dma_start(bv_sb[:], bv[:])
            ones8_sb = constp.tile([128, 2, 32], FP8)
            nc.sync.dma_start(ones8_sb[:], ones8_d[:])
            alibi_sb = constp.tile([128, B * HPC, KT], F32)
            nc.sync.dma_start(alibi_sb[:], alibi[:])
            ones_row_bf = constp.tile([1, 128], BF)    # bcast lhsT (K=1, M=128)
            nc.vector.memset(ones_row_bf[:], 1.0)
            # den broadcast uses 1/WD_SCALE so reciprocal yields WD_SCALE/den
            quart_row_bf = constp.tile([1, 128], BF)
            nc.vector.memset(quart_row_bf[:], 1.0 / WD_SCALE)

            # persistent per-core activations
            qT_sb = qkvp.tile([128, HPC, BS], BF)      # [d, hi, ss]
            kT_sb = qkvp.tile([128, HPC, BS], BF)
            v_sb = qkvp.tile([128, BS // 128, CI], FP8)  # [ss%128, ss//128, i]

            # ---- phase 1: QKV projections (bf16) ----
            with (
                tc.tile_pool(name="hsb", bufs=2) as hsp,
                tc.tile_pool(name="hs8b", bufs=2) as hs8p,
                tc.tile_pool(name="p1psum", bufs=4,
                             space=bass.MemorySpace.PSUM) as p1p,
            ):
                for ch in range(N_CHUNKS):
                    s0 = ch * SS_CHUNK
                    hsb = hsp.tile([128, JT, SS_CHUNK], BF, name="hsb")
                    hs8b = hs8p.tile([128, JT, SS_CHUNK], FP8, name="hs8b")
                    # chunk 0 split scalar/gpsimd so no queue is >1MB
                    # deep before the first matmul's inputs land
                    if ch == 0:
                        nc.scalar.dma_start(hsb[:, :JT // 2],
                                            hsT[:, ch, :JT // 2])
                        nc.gpsimd.dma_start(hsb[:, JT // 2:],
                                            hsT[:, ch, JT // 2:])
                        nc.scalar.dma_start(wk_sb[:, :JT // 2],
                                            wkT[:, :JT // 2])
                        nc.scalar.dma_start(wk_sb[:, JT // 2:],
                                            wkT[:, JT // 2:])
                        nc.scalar.dma_start(hs8b[:], hs8T[:, ch])
                    else:
                        nc.gpsimd.dma_start(hsb[:, :JT // 2],
                                            hsT[:, ch, :JT // 2])
                        nc.gpsimd.dma_start(hsb[:, JT // 2:],
                                            hsT[:, ch, JT // 2:])
                        (nc.scalar if ch < 2 else nc.sync).dma_start(
                            hs8b[:], hs8T[:, ch])
                    for w_sb, b_col, o_sb, scale in (
                        (wq_sb, bq_sb, qT_sb, INV_NORM),
                        (wk_sb, bk_sb, kT_sb, 1.0),
                    ):
                        for hi in range(HPC):
                            ps = p1p.tile([128, SS_CHUNK], F32, name="ps_qk")
                            for jt in range(JT):
                                nc.tensor.matmul(
                                    ps[:],
                                    w_sb[:, jt, hi * 128:(hi + 1) * 128],
                                    hsb[:, jt, :],
                                    start=(jt == 0), stop=(jt == JT - 1))
                            nc.scalar.activation(
                                o_sb[:, hi, s0:s0 + SS_CHUNK], ps[:],
                                mybir.ActivationFunctionType.Identity,
                                bias=b_col[:, hi:hi + 1], scale=scale)
                    for st in range(SS_CHUNK // 128):
                        ps = p1p.tile([128, CI], F32, name="ps_v")
                        # bias preload (bv pre-scaled x WD_SCALE... no: x16
                        # on host to cancel the 1/16 evacuation descale)
                        nc.tensor.matmul(ps[:], ones_row_bf[:], bv_sb[:],
                                         start=True, stop=False)
                        for j in range(JT // 2):
                            nc.tensor.matmul(
                                ps[:],
                                hs8b[:, 2 * j:2 * j + 2,
                                     st * 128:(st + 1) * 128],
                                wv_sb[:, 2 * j:2 * j + 2, :],
                                start=False, stop=(j == JT // 2 - 1),
                                perf_mode=DR)
                        # 1/16 undoes the x16 host scaling of wv (and bv)
                        nc.scalar.activation(
                            v_sb[:, ch * 4 + st, :], ps[:],
                            mybir.ActivationFunctionType.Identity,
                            scale=1.0 / 16.0)

            # late consts (dense phase only)
            wd_sb = constp.tile([128, IT, CI], FP8)
            nc.sync.dma_start(wd_sb[:], wdT[:])
            bd_col = constp.tile([128, HPC], F32)
            for ci in range(HPC):
                nc.sync.dma_start(bd_col[:, ci:ci + 1],
                                  bd_f32[ci * 128:(ci + 1) * 128, :])

            # ---- phase 2+3: attention blocks + chunked AllGather ----
            with (
                tc.tile_pool(name="stp", bufs=2,
                             space=bass.MemorySpace.PSUM) as stp,
                tc.tile_pool(name="ptp", bufs=5) as ptp,
                tc.tile_pool(name="accp", bufs=1,
                             space=bass.MemorySpace.PSUM) as accp,
                tc.tile_pool(name="denp", bufs=1,
                             space=bass.MemorySpace.PSUM) as denp,
                tc.tile_pool(name="normp", bufs=4) as normp,
            ):
                LAG = 5
                pending_tail = [None]
                ctiles = {}

                def prefetch_ctiles(blk):
                    tiles = []
                    for j in range(NPAIR):
                        it_a = IT_ORDER[2 * j]
                        hi_j, rj = it_a % HPC, (it_a // HPC) // 2
                        ct2 = ctp.tile([128, 2, QBLK], FP8, name="ctile")
                        nc.gpsimd.dma_start(
                            ct2[:],
                            gath[hi_j, blk, rj * 256:(rj + 1) * 256, :]
                            .rearrange("(i p) q -> p i q", p=128))
                        tiles.append(ct2)
                    ctiles[blk] = tiles

                def flush_tail():
                    if pending_tail[0] is not None:
                        pending_tail[0]()
                        pending_tail[0] = None

                for blk in range(N_BLOCKS):
                    b, qh = divmod(blk, N_BLOCKS // B)
                    q0 = b * S + qh * QBLK
                    for hi in range(HPC):
                        bh = b * HPC + hi
                        ctx_ps = accp.tile([128, QBLK], F32, name="ctx_ps")
                        den_ps = denp.tile([128, 2, SS_CHUNK], F32,
                                           name="den_ps")
                        pts = []
                        consumed = [0]

                        def consume(p, ctx_ps=ctx_ps, den_ps=den_ps,
                                    pts=pts, b=b, hi=hi):
                            pt = pts[p]
                            for half in range(2):
                                hs_ = slice(half * SS_CHUNK,
                                            (half + 1) * SS_CHUNK)
                                # ctx^T += v_pair^T @ P_pair  (fp8 DoubleRow)
                                nc.tensor.matmul(
                                    ctx_ps[:, hs_],
                                    v_sb[:, (b * S) // 128 + 2 * p:
                                         (b * S) // 128 + 2 * p + 2,
                                         hi * 128:(hi + 1) * 128],
                                    pt[:, :, half, :],
                                    start=(p == 0), stop=(p == NPAIR - 1),
                                    perf_mode=DR)
                                # den += ones^T @ P_pair
                                nc.tensor.matmul(
                                    den_ps[:32, half, :],
                                    ones8_sb[:],
                                    pt[:, :, half, :],
                                    start=(p == 0), stop=(p == NPAIR - 1),
                                    perf_mode=DR)

                        for kt in range(KT):
                            k0 = b * S + kt * 128
                            st_ps = stp.tile([128, 2, SS_CHUNK], F32,
                                             name="st_ps")
                            for half in range(2):
                                nc.tensor.matmul(
                                    st_ps[:, half, :],
                                    kT_sb[:, hi, k0:k0 + 128],
                                    qT_sb[:, hi,
                                          q0 + half * SS_CHUNK:
                                          q0 + (half + 1) * SS_CHUNK],
                                    start=True, stop=True)
                            if kt % 2 == 0:
                                pt = ptp.tile([128, 2, 2, SS_CHUNK], FP8,
                                              name="pt")
                                pts.append(pt)
                            # q pre-scaled by INV_NORM in phase 1; bias is
                            # alibi (per key-position partition) - EXP_SHIFT
                            nc.scalar.activation(
                                pts[-1][:, kt % 2], st_ps[:],
                                mybir.ActivationFunctionType.Exp,
                                bias=alibi_sb[:, bh, kt:kt + 1])
                            if kt == 2:
                                flush_tail()
                            while consumed[0] < (kt - LAG + 1) // 2:
                                consume(consumed[0])
                                consumed[0] += 1
                        while consumed[0] < NPAIR:
                            consume(consumed[0])
                            consumed[0] += 1
                        # den PSUM -> SBUF right away so the den bank can be
                        # reused by the next group without waiting the tail
                        den_sb = normp.tile([1, QBLK], BF, name="den_sb")
                        nc.vector.tensor_copy(
                            den_sb[:],
                            den_ps[:1, :, :].rearrange("p a b -> p (a b)"))

                        def tail(ctx_ps=ctx_ps, den_sb=den_sb, blk=blk,
                                 hi=hi):
                            denb_ps = stp.tile([128, 2, SS_CHUNK], F32,
                                               name="st_ps")
                            for half in range(2):
                                nc.tensor.matmul(
                                    denb_ps[:, half, :], quart_row_bf[:],
                                    den_sb[:, half * SS_CHUNK:
                                           (half + 1) * SS_CHUNK],
                                    start=True, stop=True)
                            denb_sb = normp.tile([128, QBLK], F32,
                                                 name="denb_sb")
                            nc.vector.reciprocal_approx_fast(
                                denb_sb[:],
                                denb_ps[:].rearrange("p a b -> p (a b)"))
                            ctxn_sb = normp.tile([128, QBLK], FP8,
                                                 name="ctxn_sb")
                            nc.vector.tensor_mul(ctxn_sb[:], ctx_ps[:],
                                                 denb_sb[:])
                            nc.sync.dma_start(bounce[blk, hi], ctxn_sb[:])
                            nc.gpsimd.collective_compute(
                                "AllGather", mybir.AluOpType.bypass,
                                replica_groups=[list(range(N_CORES))],
                                ins=[bounce[blk, hi]],
                                outs=[gath[hi, blk]])
                            # ctile prefetches slot onto the gpsimd queue
                            # after the AGs they depend on (queue-ordered)
                            if hi == 1:
                                prefetch_ctiles(blk)

                        pending_tail[0] = tail
                flush_tail()

            # ---- phase 4: output projection, fp8 DoubleRow. PSUM is
            # DMA-preloaded with WD_SCALE*(residual + bd); matmuls accumulate
            # WD_SCALE*Wd @ ctxn on top; DVE applies the 1/WD_SCALE descale.
            with (
                tc.tile_pool(name="dpsum", bufs=8,
                             space=bass.MemorySpace.PSUM) as dpp,
                tc.tile_pool(name="outp", bufs=8) as outp,
            ):
                NSC = QBLK // SS_CHUNK      # 2 seq chunks per block
                for blk in range(N_BLOCKS):
                    b, qh = divmod(blk, N_BLOCKS // B)
                    q0 = b * S + qh * QBLK
                    dps = [dpp.tile([128, SS_CHUNK], F32, name="dps")
                           for _ in range(HPC * NSC)]
                    rts = []
                    for ct in range(HPC):
                        for sc in range(NSC):
                            rt = outp.tile([128, SS_CHUNK], F32, name="rt")
                            nc.sync.dma_start(
                                rt[:],
                                residT[ct * 128:(ct + 1) * 128,
                                       q0 + sc * SS_CHUNK:
                                       q0 + (sc + 1) * SS_CHUNK])
                            rts.append(rt)
                    for j in range(NPAIR):
                        ct2 = ctiles[blk][j]
                        for ct in range(HPC):
                            for sc in range(NSC):
                                nc.tensor.matmul(
                                    dps[ct * NSC + sc][:],
                                    wd_sb[:, 2 * j:2 * j + 2,
                                          ct * 128:(ct + 1) * 128],
                                    ct2[:, :, sc * SS_CHUNK:
                                        (sc + 1) * SS_CHUNK],
                                    start=(j == 0), stop=(j == NPAIR - 1),
                                    perf_mode=DR)
                    for ct in range(HPC):
                        for sc in range(NSC):
                            c0 = ct * 128
                            s0_ = q0 + sc * SS_CHUNK
                            osb = outp.tile([128, SS_CHUNK], F32,
                                            name="osb")
                            nc.scalar.activation(
                                osb[:], dps[ct * NSC + sc][:],
                                mybir.ActivationFunctionType.Identity,
                                bias=bd_col[:, ct:ct + 1],
                                scale=1.0 / (WD_SCALE * WD_SCALE))
                            nc.vector.tensor_add(
                                osb[:], osb[:], rts[ct * NSC + sc][:])
                            nc.sync.dma_start(
                                outT[c0:c0 + 128, s0_:s0_ + SS_CHUNK],
                                osb[:])

    nc.compile()
    return nc


_NC = None


def _get_nc():
    global _NC
    if _NC is None:
        _NC = _build()
    return _NC


def _pack_w(W, sl, dtype=BF16, scale=1.0, order=None):
    # [H, CI] transposed slice -> SBUF layout [128, JT, CI], contiguous
    wT = np.asarray(W, np.float32)[sl].T * scale    # [H, CI]
    w = wT.reshape(JT, 128, CI)
    if order is not None:
        w = w[order]
    return np.ascontiguousarray(w.transpose(1, 0, 2)).astype(dtype)


def _prep_in_maps(hidden_states, residual, alibi, Wq, bq, Wk, bk, Wv, bv,
                  Wd, bd):
    hs = np.ascontiguousarray(np.asarray(hidden_states, np.float32)
                              .reshape(BS, H))
    # SBUF chunk layout [128, ch, jt, s]: element = hs[ch*512+s, jt*128+p]
    hs_r = hs.reshape(N_CHUNKS, SS_CHUNK, JT, 128).transpose(3, 0, 2, 1)
    hs_pack = np.ascontiguousarray(hs_r).astype(BF16)
    hs8_pack = np.ascontiguousarray(hs_r).astype(E4M3)
    resid = np.asarray(residual, np.float32).reshape(BS, H)
    bd_f = np.asarray(bd, np.float32)
    alibi_r = np.asarray(alibi, np.float32).reshape(B, NH, S)
    ones8 = np.ones((128, 2, 32), E4M3)
    in_maps = []
    for c in range(N_CORES):
        sl = slice(c * CI, (c + 1) * CI)
        # alibi rows ordered (b, hi); EXP_SHIFT folded into the exp bias
        al = (alibi_r[:, c * HPC:(c + 1) * HPC, :].reshape(B * HPC, S)
              - EXP_SHIFT)
        # SBUF layout [p, r, kt]: al[r, kt*128+p]
        al = np.ascontiguousarray(
            al.reshape(B * HPC, KT, 128).transpose(2, 0, 1))
        residT = np.ascontiguousarray(resid[:, sl].T)
        in_maps.append({
            "hsT": hs_pack,
            "hs8T": hs8_pack,
            "wqT": _pack_w(Wq, sl),
            "wkT": _pack_w(Wk, sl),
            "wvT": _pack_w(Wv, sl, dtype=E4M3, scale=16.0),
            "wdT": _pack_w(Wd, sl, dtype=E4M3, scale=WD_SCALE,
                           order=IT_ORDER),
            "bq": np.asarray(bq, np.float32)[sl].reshape(CI, 1),
            "bk": np.asarray(bk, np.float32)[sl].reshape(CI, 1),
            "bv": (np.asarray(bv, np.float32)[sl].reshape(1, CI)
                   * 16.0).astype(BF16),
            "ones8": ones8,
            "bd": bd_f[sl].reshape(CI, 1),
            "alibi": al,
            "residT": residT,
        })
    return in_maps


def run(trace=False, trace_cores=None, stitch_traces=False, **inputs):
    nc = _get_nc()
    in_maps = _prep_in_maps(**inputs)
    res = bass_utils.run_bass_kernel_spmd(
        nc, in_maps, core_ids=list(range(N_CORES)), trace=trace,
        trace_cores=trace_cores, stitch_traces=stitch_traces)
    full = np.empty((BS, H), np.float32)
    for c in range(N_CORES):
        full[:, c * CI:(c + 1) * CI] = res.results[c]["outT"].T
    return full.reshape(B, S, H), res


def kernel(**inputs):
    out, _ = run(trace=False, **inputs)
    return out
